# revision 1
# baseline (speedup 1.0000x reference)
"""Trainium2 Bass kernel for nn_ChebyshevLayer (gnn_message_passing).

Strategy (8 NeuronCores, SPMD):
- X0 = transpose(x,(1,2,0)).reshape(M, FIN*N) -> [50000, 128]; pad rows to 50176.
- Rows are dealt to cores by (qA,qB) "class cell" so every core has an identical
  reduce-segment structure (one shared NEFF). qA/qB = ceil(row nnz count in each
  column-half / 4); the column space is split in half so gather indices fit int16.
- Each SpMM launch: per-core transposed HBM dma_gather of X[col] rows (bf16,
  256B elements), partition_broadcast of vals, DVE multiply, strided DVE
  segment reduces into Y^T [128=(f,n), rows], then X_next = alpha*Y - X_prev.
- Host concatenates per-core row slices between the 3 SpMM launches (pure data
  movement) and re-feeds the full X as the next gather source.
- Final launch: einsum as 4 accumulated PE matmuls with block-diagonal W-hat
  plus a K=1 ones-matmul adding the bias.
"""

import numpy as np
import ml_dtypes

import concourse.bacc as bacc
import concourse.mybir as mybir
from concourse.tile import TileContext
from concourse.bass_utils import run_bass_kernel_spmd
from concourse.library_config import mlp

N, M, FIN, FOUT, KCH = 8, 50000, 16, 32, 4
MP = 50176               # padded row count (392*128)
HALF = MP // 2           # 25088 (int16 gather index limit per half)
PW = 4                   # class pad width
CHUNK = 4096             # gather slots per chunk
NCORE = 8
BF16 = ml_dtypes.bfloat16


def _ceil(a, b):
    return -(-a // b)


class Plan:
    pass


def build_plan(rows, cols):
    """Shared (all-core) chunk/segment plan + per-core row assignment."""
    p = Plan()
    cA = np.bincount(rows[cols < HALF], minlength=MP).astype(np.int64)
    cB = np.bincount(rows[cols >= HALF], minlength=MP).astype(np.int64)
    qA = _ceil(cA, PW)
    qB = _ceil(cB, PW)
    cell = qA * 64 + qB
    order = np.argsort(cell, kind="stable")
    cell_s = cell[order]
    ucells, starts_u, Ncell = np.unique(cell_s, return_index=True, return_counts=True)
    Pcell = _ceil(Ncell, NCORE)
    Rp = int(Pcell.sum())
    R128 = _ceil(Rp, 128) * 128
    cell_base = np.concatenate([[0], np.cumsum(Pcell)[:-1]])

    dev2glob = np.full((NCORE, Rp), -1, np.int64)
    j_of = np.repeat(np.arange(len(ucells)), Ncell)
    i_of = np.arange(len(order)) - np.repeat(starts_u, Ncell)
    dev2glob[i_of % NCORE, cell_base[j_of] + i_of // NCORE] = order

    qA_c = (ucells // 64).astype(np.int64)
    qB_c = (ucells % 64).astype(np.int64)

    # chunks: list of dicts {half, segs:[(soff, n, w, yoff, write)]}
    chunks = []
    base = [np.zeros(Rp, np.int64), np.zeros(Rp, np.int64)]  # per-half slot base per dev row
    wdev = [np.repeat(qA_c * PW, Pcell), np.repeat(qB_c * PW, Pcell)]
    for half in (0, 1):
        cur = CHUNK  # force new chunk at start of each half
        ch = None
        for j in range(len(ucells)):
            w = int((qA_c if half == 0 else qB_c)[j]) * PW
            if w == 0:
                continue
            wr = True if half == 0 else (qA_c[j] == 0)
            r = int(cell_base[j])
            rem = int(Pcell[j])
            while rem > 0:
                nfit = (CHUNK - cur) // w
                if nfit == 0:
                    ch = {"half": half, "segs": []}
                    chunks.append(ch)
                    cur = 0
                    nfit = CHUNK // w
                n = min(rem, nfit)
                ch["segs"].append((cur, n, w, r, wr))
                gslot = (len(chunks) - 1) * CHUNK + cur
                base[half][r:r + n] = gslot + np.arange(n) * w
                cur += n * w
                r += n
                rem -= n
    p.chunks = chunks
    p.nch = len(chunks)
    p.stot = p.nch * CHUNK
    p.Rp, p.R128 = Rp, R128
    p.dev2glob = dev2glob
    p.baseA, p.baseB = base[0], base[1]
    return p


def build_core_inputs(p, k, rows, cols, vals):
    """Per-core idx/vals streams following the shared plan."""
    g2d = np.full(MP, -1, np.int64)
    mine = p.dev2glob[k] >= 0
    g2d[p.dev2glob[k][mine]] = np.nonzero(mine)[0]
    idx_arr = np.zeros(p.stot, np.int16)
    val_arr = np.zeros(p.stot, np.float32)
    for half in (0, 1):
        m = ((cols < HALF) if half == 0 else (cols >= HALF)) & (g2d[rows] >= 0)
        r, c, v = rows[m], cols[m], vals[m]
        o = np.argsort(r, kind="stable")
        r, c, v = r[o], c[o], v[o]
        first = np.searchsorted(r, r, side="left")
        rank = np.arange(len(r)) - first
        basearr = p.baseA if half == 0 else p.baseB
        pos = basearr[g2d[r]] + rank
        idx_arr[pos] = (c if half == 0 else c - HALF).astype(np.int16)
        val_arr[pos] = v
    idx_w = np.tile(
        np.hstack([idx_arr[i * CHUNK:(i + 1) * CHUNK].reshape(-1, 16).T
                   for i in range(p.nch)]), (8, 1)).astype(np.int16)
    vals_t = np.tile(val_arr.astype(BF16).reshape(1, p.stot), (128, 1))
    return idx_w, vals_t


def build_spmm_nc(p, alpha=2.0, reps=1):
    nc = bacc.Bacc("TRN2")
    dt = mybir.dt
    xsrc = nc.dram_tensor("xsrc", [MP, 128], dt.bfloat16, kind="ExternalInput")
    idx = nc.dram_tensor("idx", [128, p.stot // 16], dt.int16, kind="ExternalInput")
    valst = nc.dram_tensor("vals", [128, p.stot], dt.bfloat16, kind="ExternalInput")
    xprev = nc.dram_tensor("xprev", [128, p.R128], dt.float32, kind="ExternalInput")
    xnext = nc.dram_tensor("xnext", [128, p.R128], dt.float32, kind="ExternalOutput")
    with TileContext(nc) as tc:
        nc.gpsimd.load_library(mlp)
        with tc.tile_pool(name="io", bufs=1) as iop, \
             tc.tile_pool(name="g", bufs=2) as gp, \
             tc.tile_pool(name="vb", bufs=2) as vbp, \
             tc.tile_pool(name="tmp", bufs=2) as tp, \
             tc.tile_pool(name="y", bufs=1) as yp:
            idx_sb = iop.tile([128, p.stot // 16], dt.int16)
            nc.sync.dma_start(idx_sb[:], idx[:])
            xp_sb = iop.tile([128, p.R128], dt.float32)
            nc.sync.dma_start(xp_sb[:], xprev[:])
            for _ in range(reps):
                Y = yp.tile([128, p.R128], dt.float32, tag="Y")
                nc.vector.memset(Y[:], 0.0)
                for ci, ch in enumerate(p.chunks):
                    g = gp.tile([128, 1, CHUNK], dt.bfloat16, tag="g")
                    src = xsrc[0:HALF, :] if ch["half"] == 0 else xsrc[HALF:MP, :]
                    nc.gpsimd.dma_gather(
                        g[:], src, idx_sb[:, ci * CHUNK // 16:(ci + 1) * CHUNK // 16],
                        CHUNK, CHUNK, 128, transpose=True, single_packet=False)
                    vb = vbp.tile([128, CHUNK], dt.bfloat16, tag="vb")
                    nc.sync.dma_start(vb[:], valst[:, ci * CHUNK:(ci + 1) * CHUNK])
                    g2 = g[:].rearrange("p a c -> p (a c)")
                    nc.vector.tensor_tensor(g2, g2, vb[:], mybir.AluOpType.mult)
                    for (soff, n, w, yoff, wr) in ch["segs"]:
                        gr = g[:, :, soff:soff + n * w].rearrange(
                            "p a (n w) -> p (a n) w", w=w)
                        if wr:
                            nc.vector.tensor_reduce(
                                Y[:, yoff:yoff + n], gr,
                                mybir.AxisListType.X, mybir.AluOpType.add)
                        else:
                            t = tp.tile([128, n], dt.float32, tag="t")
                            nc.vector.tensor_reduce(
                                t[:], gr, mybir.AxisListType.X, mybir.AluOpType.add)
                            nc.vector.tensor_tensor(
                                Y[:, yoff:yoff + n], Y[:, yoff:yoff + n], t[:],
                                mybir.AluOpType.add)
                nc.vector.tensor_scalar(Y[:], Y[:], float(alpha), None,
                                        mybir.AluOpType.mult)
                xn = yp.tile([128, p.R128], dt.float32, tag="xn")
                nc.vector.tensor_tensor(xn[:], Y[:], xp_sb[:],
                                        mybir.AluOpType.subtract)
                nc.sync.dma_start(xnext[:, :], xn[:])
    nc.compile()
    return nc


def build_einsum_nc(R128, reps=1):
    from contextlib import ExitStack
    nc = bacc.Bacc("TRN2")
    dt = mybir.dt
    ts = [nc.dram_tensor(f"t{i}", [128, R128], dt.bfloat16, kind="ExternalInput")
          for i in range(KCH)]
    wm = nc.dram_tensor("wm", [KCH * 128, 256], dt.bfloat16, kind="ExternalInput")
    onesb = nc.dram_tensor("onesb", [1, 128], dt.bfloat16, kind="ExternalInput")
    bvec = nc.dram_tensor("bvec", [1, 256], dt.bfloat16, kind="ExternalInput")
    outt = nc.dram_tensor("outt", [R128, 256], dt.float32, kind="ExternalOutput")
    ntile = R128 // 128
    with TileContext(nc) as tc:
        with tc.tile_pool(name="io", bufs=1) as iop, \
             tc.tile_pool(name="ps", bufs=4, space="PSUM") as psp, \
             tc.tile_pool(name="o", bufs=4) as osp:
            t_sb = []
            for i in range(KCH):
                tt = iop.tile([128, R128], dt.bfloat16, tag=f"t{i}")
                nc.sync.dma_start(tt[:], ts[i][:])
                t_sb.append(tt)
            w_sb = []
            for i in range(KCH):
                wt = iop.tile([128, 256], dt.bfloat16, tag=f"w{i}")
                nc.sync.dma_start(wt[:], wm[i * 128:(i + 1) * 128, :])
                w_sb.append(wt)
            on_sb = iop.tile([1, 128], dt.bfloat16)
            nc.sync.dma_start(on_sb[:], onesb[:])
            bv_sb = iop.tile([1, 256], dt.bfloat16)
            nc.sync.dma_start(bv_sb[:], bvec[:])
            for _ in range(reps):
                for t in range(ntile):
                    ps = psp.tile([128, 256], dt.float32, tag="ps")
                    for k in range(KCH):
                        nc.tensor.matmul(ps[:],
                                         t_sb[k][:, t * 128:(t + 1) * 128],
                                         w_sb[k][:], start=(k == 0), stop=False)
                    nc.tensor.matmul(ps[:], on_sb[:], bv_sb[:],
                                     start=False, stop=True)
                    o = osp.tile([128, 256], dt.float32, tag="o")
                    nc.vector.tensor_copy(o[:], ps[:])
                    nc.sync.dma_start(outt[t * 128:(t + 1) * 128, :], o[:])
    nc.compile()
    return nc


_CACHE = {}


def _run(nc, in_maps):
    return run_bass_kernel_spmd(nc, in_maps, core_ids=list(range(NCORE)))


def kernel(x, l_vals, w, b, l_row, l_col, _timing=None):
    x = np.asarray(x, np.float32)
    l_vals = np.asarray(l_vals, np.float32)
    w = np.asarray(w, np.float32)
    b = np.asarray(b, np.float32)
    rows = np.asarray(l_row).astype(np.int64)
    cols = np.asarray(l_col).astype(np.int64)

    p = build_plan(rows, cols)
    key = (p.nch, p.R128)
    if key not in _CACHE:
        _CACHE[key] = (build_spmm_nc(p, 1.0), build_spmm_nc(p, 2.0), build_einsum_nc(p.R128))
    nc_spmm1, nc_spmm2, nc_ein = _CACHE[key]

    # X0 full [MP, 128] f32 (rows padded with zeros)
    X0 = np.zeros((MP, 128), np.float32)
    X0[:M] = x.transpose(1, 2, 0).reshape(M, FIN * N)

    core_in = [build_core_inputs(p, k, rows, cols, l_vals) for k in range(NCORE)]

    # per-core X^T slices in device order (virtual rows -> 0)
    def dev_slices(Xfull):
        out = []
        for k in range(NCORE):
            s = np.zeros((128, p.R128), np.float32)
            mine = p.dev2glob[k] >= 0
            s[:, :p.Rp][:, mine] = Xfull[p.dev2glob[k][mine]].T
            out.append(s)
        return out

    def assemble(slices):
        Xf = np.zeros((MP, 128), np.float32)
        for k in range(NCORE):
            mine = p.dev2glob[k] >= 0
            Xf[p.dev2glob[k][mine]] = slices[k][:, :p.Rp][:, mine].T
        return Xf

    Xt_slices = [dev_slices(X0)]          # T0 slices
    Xcur = X0
    zeros_sl = [np.zeros((128, p.R128), np.float32)] * NCORE

    times = []
    import time
    for it in range(KCH - 1):
        ncs = nc_spmm1 if it == 0 else nc_spmm2
        xprev_sl = zeros_sl if it == 0 else Xt_slices[it - 1]
        in_maps = [{
            "xsrc": Xcur.astype(BF16),
            "idx": core_in[k][0],
            "vals": core_in[k][1],
            "xprev": xprev_sl[k],
        } for k in range(NCORE)]
        t0 = time.time()
        res = _run(ncs, in_maps)
        times.append(time.time() - t0)
        new_sl = [res.results[k]["xnext"] for k in range(NCORE)]
        Xt_slices.append(new_sl)
        Xcur = assemble(new_sl)

    # einsum
    wmat = np.zeros((KCH * 128, 256), np.float32)
    for k in range(KCH):
        for pp in range(128):
            f, n = pp // 8, pp % 8
            wmat[k * 128 + pp, n * 32:(n + 1) * 32] = w[f, k, :]
    bv = np.tile(b.reshape(1, FOUT), (1, 8)).astype(np.float32)
    ein_maps = [{
        **{f"t{i}": Xt_slices[i][k].astype(BF16) for i in range(KCH)},
        "wm": wmat.astype(BF16),
        "onesb": np.ones((1, 128), BF16),
        "bvec": bv.astype(BF16),
    } for k in range(NCORE)]
    t0 = time.time()
    res = _run(nc_ein, ein_maps)
    times.append(time.time() - t0)

    out = np.zeros((N, M, FOUT), np.float32)
    for k in range(NCORE):
        o = res.results[k]["outt"]            # [R128, 256]
        mine = p.dev2glob[k] >= 0
        rows_k = p.dev2glob[k][mine]
        real = rows_k < M
        o3 = o[:p.Rp][mine][real].reshape(-1, N, FOUT)   # [nrows, n, o]
        out[:, rows_k[real], :] = o3.transpose(1, 0, 2)
    if _timing is not None:
        _timing.extend(times)
    return out



# revision 6
# speedup vs baseline: 141.0021x; 141.0021x over previous
"""Trainium2 single-launch Chebyshev kernel (v2).

Design (8 cores, SPMD, one NEFF launch):
- Rows dealt by parity: even original rows at even device slots (core r%16//2...,
  see plan), so permuted-id parity == original row parity. Gather sources are
  two stride-2 row views (even/odd) of the full permuted X in HBM; int16 idx
  fits (25088 < 32768).
- Each SpMM iteration per core: dma_gather (transpose=False, 256B rows, its
  ~100k nnz exactly packed into 128-e subchunks), then one PE matmul per
  subchunk: stationary G_sub [128e x 128c], moving V_sub [128e x nslots]
  (val at one-hot window column) accumulating Y^T windows in PSUM; DVE
  copies/adds PSUM windows into Y^T [128, R128]. X_next = 2Y - X_prev (bf16),
  PE-transposed to row-major, DMA'd to HBM, AllGather'd for the next
  iteration's gather source. 3 iterations, zero host round-trips.
- Final einsum fused: per 128-row tile, 4 matmuls (stationary T_k tile,
  moving block-diag W-hat) + bias matmul -> [R128, 256] f32 out.
"""
import numpy as np
import ml_dtypes

import concourse.bacc as bacc
import concourse.mybir as mybir
from concourse.tile import TileContext
from concourse.bass_utils import run_bass_kernel_spmd
from concourse.library_config import mlp

N, FIN, FOUT, KCH = 8, 16, 32, 4
NCORE = 8
CHUNK = 8192
BF16 = ml_dtypes.bfloat16
dt = mybir.dt


class Plan2:
    pass


def build_plan2(rows, cols, M, nsub_max=2, nslots_cap=48):
    p = Plan2()
    p.M = M
    i_even = np.arange(0, M, 2) // 2
    i_odd = (np.arange(1, M, 2) - 1) // 2
    owner = np.empty(M, np.int64)
    slot = np.empty(M, np.int64)
    owner[0::2] = i_even % NCORE
    slot[0::2] = 2 * (i_even // NCORE)
    owner[1::2] = i_odd % NCORE
    slot[1::2] = 2 * (i_odd // NCORE) + 1
    npar = -(-(M // 2) // NCORE)
    R128 = -(-2 * npar // 128) * 128
    p.R128 = R128
    p.RTOT = NCORE * R128
    p.owner, p.slot = owner, slot
    p.pid = owner * R128 + slot

    colpar = (cols % 2).astype(np.int64)
    W = np.zeros((2, NCORE, R128), np.int64)
    np.add.at(W, (colpar, owner[rows], slot[rows]), 1)
    p.W = W

    # greedy groups per view; then pack groups into chunks (no chunk straddle)
    # sched[view] = list of chunks; chunk = list of (sub0, nsub, slot_lo, nslots, voff)
    p.sched = {0: [], 1: []}
    p.nch = {}
    voff = 0
    for v in (0, 1):
        groups = []
        s = 0
        while s < R128:
            cum = np.zeros(NCORE, np.int64)
            n = 0
            while s + n < R128 and n < nslots_cap:
                c2 = cum + W[v, :, s + n]
                if c2.max() > 128 * nsub_max:
                    break
                cum = c2
                n += 1
            if n == 0:
                n = 1
                cum = W[v, :, s].copy()
            nsub = max(1, -(-int(cum.max()) // 128))
            groups.append((s, n, nsub))
            s += n
        # pack into chunks (exact sizes: last/each chunk only as big as used)
        chunks = [[]]
        sub_used = 0
        subs_per_chunk = []
        for (s0, n, nsub) in groups:
            if sub_used + nsub > CHUNK // 128:
                subs_per_chunk.append(sub_used)
                chunks.append([])
                sub_used = 0
            chunks[-1].append((sub_used, nsub, s0, n, voff))
            sub_used += nsub
            voff += nsub * n
        subs_per_chunk.append(sub_used)
        p.sched[v] = chunks
        p.nch[v] = len(chunks)
        p.chunk_esz = getattr(p, "chunk_esz", {})
        p.chunk_esz[v] = [s * 128 for s in subs_per_chunk]
    p.NV = voff
    p.stot = {v: sum(p.chunk_esz[v]) for v in (0, 1)}
    return p


def build_core_inputs2(p, k, rows, cols, vals):
    """idx stream (int16, 16-wrapped per chunk, E chunks then O chunks) and
    V matrix [128, NV] float32 for core k. Vectorized."""
    R128 = p.R128
    mine = p.owner[rows] == k
    c = cols[mine]
    v = vals[mine]
    sl = p.slot[rows[mine]]
    cp = (c % 2).astype(np.int64)
    gidx = (p.pid[c] // 2).astype(np.int64)

    Vm = np.zeros((128, p.NV), np.float32)
    idx_parts = []
    for view in (0, 1):
        m = cp == view
        sl_v, gi_v, vv = sl[m], gidx[m], v[m]
        o = np.argsort(sl_v, kind="stable")
        sl_v, gi_v, vv = sl_v[o], gi_v[o], vv[o]
        widths = p.W[view, k]
        starts = np.concatenate([[0], np.cumsum(widths)[:-1]])
        slot_epos = np.zeros(R128, np.int64)
        slot_gbase = np.zeros(R128, np.int64)
        slot_voff = np.zeros(R128, np.int64)
        slot_n = np.ones(R128, np.int64)
        slot_j = np.zeros(R128, np.int64)
        cbases = np.concatenate([[0], np.cumsum(p.chunk_esz[view])[:-1]])
        for ci, chunk in enumerate(p.sched[view]):
            for (sub0, nsub, s0, n, voff) in chunk:
                ebase = int(cbases[ci]) + sub0 * 128
                wseg = widths[s0:s0 + n]
                cums = np.concatenate([[0], np.cumsum(wseg)[:-1]])
                slot_epos[s0:s0 + n] = ebase + cums
                slot_gbase[s0:s0 + n] = ebase
                slot_voff[s0:s0 + n] = voff
                slot_n[s0:s0 + n] = n
                slot_j[s0:s0 + n] = np.arange(n)
        rank = np.arange(len(sl_v)) - starts[sl_v]
        epos = slot_epos[sl_v] + rank
        idx_arr = np.zeros(p.stot[view], np.int16)
        idx_arr[epos] = gi_v.astype(np.int16)
        vcol = (slot_voff[sl_v]
                + ((epos - slot_gbase[sl_v]) // 128) * slot_n[sl_v]
                + slot_j[sl_v])
        Vm[epos % 128, vcol] = vv
        idx_parts.append(idx_arr)
    idx_all = np.concatenate(idx_parts)
    blocks = []
    off = 0
    for v in (0, 1):
        for esz in p.chunk_esz[v]:
            blocks.append(idx_all[off:off + esz].reshape(-1, 16).T)
            off += esz
    idx_w = np.tile(np.concatenate(blocks, axis=1), (8, 1)).astype(np.int16)
    return idx_w, Vm


def build_cheb_nc(p, reps=1, debug=False, hwreps=1):
    R128, RTOT, NV = p.R128, p.RTOT, p.NV
    NT = R128 // 128
    nc = bacc.Bacc("TRN2", num_devices=NCORE, num_swdge_queues=4)
    x0full = nc.dram_tensor("x0full", [RTOT, 128], dt.bfloat16, kind="ExternalInput")
    t0sl = nc.dram_tensor("t0sl", [128, R128], dt.bfloat16, kind="ExternalInput")
    idx = nc.dram_tensor("idx", [128, (p.stot[0] + p.stot[1]) // 16], dt.int16,
                         kind="ExternalInput")
    vmat = nc.dram_tensor("vmat", [128, NV], dt.bfloat16, kind="ExternalInput")
    wmat = nc.dram_tensor("wmat", [KCH * 128, 256], dt.bfloat16, kind="ExternalInput")
    onesb = nc.dram_tensor("onesb", [1, 128], dt.bfloat16, kind="ExternalInput")
    bvec = nc.dram_tensor("bvec", [1, 256], dt.bfloat16, kind="ExternalInput")
    ident = nc.dram_tensor("ident", [128, 128], dt.bfloat16, kind="ExternalInput")
    outt = nc.dram_tensor("outt", [R128, 256], dt.float32, kind="ExternalOutput")
    if debug:
        tdbg = [nc.dram_tensor(f"t{i}o", [128, R128], dt.bfloat16,
                               kind="ExternalOutput") for i in (1, 2, 3)]
    with TileContext(nc) as tc:
        nc.gpsimd.load_library(mlp)
        with tc.tile_pool(name="dram", bufs=2, space="DRAM") as drp, \
             tc.tile_pool(name="io", bufs=1) as iop, \
             tc.tile_pool(name="g", bufs=2) as gp, \
             tc.tile_pool(name="ps", bufs=2, space="PSUM") as psp, \
             tc.tile_pool(name="pst", bufs=2, space="PSUM") as ptp, \
             tc.tile_pool(name="pse", bufs=2, space="PSUM") as psep, \
             tc.tile_pool(name="y", bufs=1) as yp, \
             tc.tile_pool(name="rm", bufs=1) as rmp, \
             tc.tile_pool(name="o", bufs=4) as osp:
            idx_sb = iop.tile([128, (p.stot[0] + p.stot[1]) // 16], dt.int16, name="idx_sb")
            nc.sync.dma_start(idx_sb[:], idx[:])
            vm_sb = iop.tile([128, NV], dt.bfloat16, name="vm_sb")
            nc.sync.dma_start(vm_sb[:], vmat[:])
            t_sb = [iop.tile([128, R128], dt.bfloat16, tag=f"T{i}", name=f"T{i}") for i in range(KCH)]
            nc.sync.dma_start(t_sb[0][:], t0sl[:])
            w_sb = []
            for i in range(KCH):
                wt = iop.tile([128, 256], dt.bfloat16, tag=f"w{i}", name=f"w{i}")
                nc.sync.dma_start(wt[:], wmat[i * 128:(i + 1) * 128, :])
                w_sb.append(wt)
            on_sb = iop.tile([1, 128], dt.bfloat16, name="on_sb")
            nc.sync.dma_start(on_sb[:], onesb[:])
            bv_sb = iop.tile([1, 256], dt.bfloat16, name="bv_sb")
            nc.sync.dma_start(bv_sb[:], bvec[:])
            id_sb = iop.tile([128, 128], dt.bfloat16, name="id_sb")
            nc.sync.dma_start(id_sb[:], ident[:])

            from contextlib import ExitStack

            def rep_body():
                agb_prev = None
                gctr = [0]
                for it in range(KCH - 1):
                    src = x0full if it == 0 else agb_prev
                    views = [src[0::2, :], src[1::2, :]]
                    Y = yp.tile([128, R128], dt.float32, tag="Y", name="Y")
                    for v in (0, 1):
                        chbase = 0 if v == 0 else p.nch[0]
                        ibase = 0 if v == 0 else p.stot[0] // 16
                        icbases = [0]
                        for esz in p.chunk_esz[v][:-1]:
                            icbases.append(icbases[-1] + esz // 16)
                        flush_ps = None
                        flush_cols = 0
                        flush_slot0 = 0
                        pending = []  # (ps tile, slot_lo, ncols)

                        def flush(ps, slot0, ncols, view):
                            if ncols == 0:
                                return
                            if view == 0:
                                nc.vector.tensor_copy(
                                    Y[:, slot0:slot0 + ncols], ps[:, 0:ncols])
                            else:
                                nc.vector.tensor_tensor(
                                    Y[:, slot0:slot0 + ncols],
                                    Y[:, slot0:slot0 + ncols], ps[:, 0:ncols],
                                    mybir.AluOpType.add)

                        for ci, chunk in enumerate(p.sched[v]):
                            esz = p.chunk_esz[v][ci]
                            g = gp.tile([128, esz // 128, 128], dt.bfloat16, tag="g", name="g")
                            ic = ibase + icbases[ci]
                            nc.gpsimd.dma_gather(
                                g[:], views[v], idx_sb[:, ic:ic + esz // 16],
                                esz, esz, 128, elem_step=256,
                                transpose=False, single_packet=False,
                                queue_num=gctr[0] % 4)
                            gctr[0] += 1
                            for (sub0, nsub, s0, n, voff) in chunk:
                                if flush_ps is None or flush_cols + n > 512:
                                    if flush_ps is not None:
                                        flush(flush_ps, flush_slot0, flush_cols, v)
                                    flush_ps = psp.tile([128, 512], dt.float32, tag="ps", name="psw")
                                    flush_cols = 0
                                    flush_slot0 = s0
                                for s in range(nsub):
                                    nc.tensor.matmul(
                                        flush_ps[:, flush_cols:flush_cols + n],
                                        g[:, sub0 + s, :],
                                        vm_sb[:, voff + s * n:voff + (s + 1) * n],
                                        start=(s == 0), stop=(s == nsub - 1),
                                        skip_group_check=True)
                                flush_cols += n
                        if flush_ps is not None:
                            flush(flush_ps, flush_slot0, flush_cols, v)

                    # X_next = (2)Y - Tprev
                    tn = t_sb[it + 1]
                    if it == 0:
                        nc.vector.tensor_copy(tn[:], Y[:])
                    else:
                        nc.vector.tensor_scalar(Y[:], Y[:], 2.0, None,
                                                mybir.AluOpType.mult)
                        nc.vector.tensor_tensor(tn[:], Y[:], t_sb[it - 1][:],
                                                mybir.AluOpType.subtract)
                    if debug:
                        nc.sync.dma_start(tdbg[it][:], tn[:])
                    if it < KCH - 2:
                        # transpose to row-major and allgather
                        rm = rmp.tile([128, NT, 128], dt.bfloat16, tag="rm", name="rm")
                        for t in range(NT):
                            pt = ptp.tile([128, 128], dt.bfloat16, tag="pt", name="pt")
                            nc.tensor.transpose(pt[:], tn[:, t * 128:(t + 1) * 128],
                                                id_sb[:])
                            nc.vector.tensor_copy(rm[:, t, :], pt[:])
                        slc = drp.tile([R128, 128], dt.bfloat16, tag="slc", name="slc")
                        agb = drp.tile([RTOT, 128], dt.bfloat16, tag="agb", name="agb",
                                       addr_space="Shared")
                        dview = slc[:].rearrange("(t q) c -> q t c", q=128)
                        nc.sync.dma_start(dview, rm[:])
                        nc.gpsimd.collective_compute(
                            "AllGather", mybir.AluOpType.bypass,
                            replica_groups=[list(range(NCORE))],
                            ins=[slc.opt()],
                            outs=[agb.opt()],
                        )
                        agb_prev = agb

                # einsum
                for t in range(NT):
                    ps = psep.tile([128, 256], dt.float32, tag="pse", name="pse")
                    for kk in range(KCH):
                        nc.tensor.matmul(ps[:], t_sb[kk][:, t * 128:(t + 1) * 128],
                                         w_sb[kk][:], start=(kk == 0), stop=False)
                    nc.tensor.matmul(ps[:], on_sb[:], bv_sb[:],
                                     start=False, stop=True)
                    o = osp.tile([128, 256], dt.float32, tag="o", name="oe")
                    nc.vector.tensor_copy(o[:], ps[:])
                    nc.sync.dma_start(outt[t * 128:(t + 1) * 128, :], o[:])

            if hwreps > 1:
                with tc.For_i(0, hwreps) as _i:
                    rep_body()
            else:
                for _rep in range(reps):
                    rep_body()
    nc.compile()
    return nc


def build_host_inputs(p, x, w, b, rows, cols, vals):
    """Shared inputs + per-core idx/V/t0."""
    M, R128 = p.M, p.R128
    n, m, fin = x.shape
    X0 = x.transpose(1, 2, 0).reshape(m, fin * n).astype(np.float32)
    X0p = np.zeros((p.RTOT, 128), np.float32)
    X0p[p.pid[:M]] = X0
    x0full = X0p.astype(BF16)

    wm = np.zeros((KCH * 128, 256), np.float32)
    for kk in range(KCH):
        for pp in range(128):
            f, nn = pp // 8, pp % 8
            wm[kk * 128 + pp, nn * 32:(nn + 1) * 32] = w[f, kk, :]
    bv = np.tile(b.reshape(1, FOUT), (1, 8)).astype(np.float32)

    core = []
    for k in range(NCORE):
        idx_w, Vm = build_core_inputs2(p, k, rows, cols, vals)
        t0 = np.zeros((128, R128), np.float32)
        mine = p.owner[np.arange(M)] == k
        t0[:, p.slot[np.arange(M)[mine]]] = X0[mine].T
        core.append({
            "x0full": x0full,
            "t0sl": t0.astype(BF16),
            "idx": idx_w,
            "vmat": Vm.astype(BF16),
            "wmat": wm.astype(BF16),
            "onesb": np.ones((1, 128), BF16),
            "bvec": bv.astype(BF16),
            "ident": np.eye(128, dtype=np.float32).astype(BF16),
        })
    return core


def assemble_out(p, results):
    M = p.M
    out = np.zeros((N, M, FOUT), np.float32)
    rows_all = np.arange(M)
    for k in range(NCORE):
        o = results[k]["outt"]            # [R128, 256]
        mine = p.owner[rows_all] == k
        rk = rows_all[mine]
        o3 = o[p.slot[rk]].reshape(-1, N, FOUT)
        out[:, rk, :] = o3.transpose(1, 0, 2)
    return out


_CACHE = {}


def kernel(x, l_vals, w, b, l_row, l_col, _timing=None, _reps=1, _debug=False,
           _hwreps=1):
    x = np.asarray(x, np.float32)
    l_vals = np.asarray(l_vals, np.float32)
    w = np.asarray(w, np.float32)
    b = np.asarray(b, np.float32)
    rows = np.asarray(l_row).astype(np.int64)
    cols = np.asarray(l_col).astype(np.int64)
    M = x.shape[1]

    p = build_plan2(rows, cols, M)
    key = (M, p.NV, p.nch[0], p.nch[1], _reps, _debug, _hwreps)
    if key not in _CACHE:
        _CACHE[key] = build_cheb_nc(p, reps=_reps, debug=_debug, hwreps=_hwreps)
    nc = _CACHE[key]
    core_in = build_host_inputs(p, x, w, b, rows, cols, vals=l_vals)
    import time
    t0 = time.time()
    res = run_bass_kernel_spmd(nc, core_in, core_ids=list(range(NCORE)))
    if _timing is not None:
        _timing.append(time.time() - t0)
    out = assemble_out(p, res.results)
    if _debug:
        out = (out, res)
    return out
